# revision 40
# baseline (speedup 1.0000x reference)
"""Trainium2 Bass kernel for nn_AugmentedAttentionHead.

Data-parallel over batch (8/core); transposed softmax ([k-part, q-free]) so
the exp() output feeds attn@v directly as the stationary operand; log-space
Gaussian bias reconstructed by one selector matmul per k-chunk, restricted
to a |t_q - t_k| < 96 window outside which the bias is < e^-8 * alpha
(sigmoid sigma <= 1 and the 24x24 grid make farther pairs negligible);
x host-cast to bf16 and pre-transposed (halves input DMA); bf16 matmul
operands everywhere (1 cyc/row at any PE width); the 1/sqrt(H) softmax
scale is applied via the final exp's scale parameter; all 5 k-chunks' tail
columns [512:578) accumulate in one shared PSUM bank and are finished by a
single exp.

Elementwise work is split between the DVE and the otherwise-idle Pool
(gpsimd) engine (Pool cannot touch PSUM, so all PSUM evacuations stay on
DVE); consts arrive in two packed DMAs and x in one DMA per batch to cut
HWDGE serialization.

The per-batch work is split into phases pipelined along anti-diagonals
across batches; within each slot the active phases are emitted round-robin
at chunk granularity so each engine's in-order queue alternates between
independent batches (blocked instructions park in the 4-deep wait queue
while ready ones behind them dispatch). x is prefetched one slot ahead.
"""

import numpy as np
import ml_dtypes
from contextlib import ExitStack

B, T, E, H = 64, 577, 768, 64
GRID = 24
EPS = 1e-5
NCORES = 8
BPC = B // NCORES

TSZ = [128, 128, 128, 128, 65]
TOFF = [0, 128, 256, 384, 512]
NCH = 5
FW = 50
NEG = -1.0e30
NPH = 5

# Gaussian-bias windows: |t_q - t_k| < 96 covers every pair with
# dy^2+dx^2 < 16; outside, the bias is <= alpha*e^-8 (negligible).
QLO = [max(0, TOFF[c] - 96) for c in range(NCH)]
QHI = [min(578, TOFF[c] + TSZ[c] + 96) for c in range(NCH)]
WAE = [min(QHI[c], 512) for c in range(NCH)]   # window end within bank A
WB = [WAE[c] - QLO[c] for c in range(NCH)]     # bank-A window width
TAILB = [QHI[c] > 512 for c in range(NCH)]     # bias extends into [512,578)

BF16 = ml_dtypes.bfloat16

# packed bf16 const blob column offsets
OFF_WEXT = 0            # [128, 6*192]
OFF_WSA = 1152          # [64, 4]
OFF_SEL = 1156          # [50, 5*128]
OFF_ID = 1796           # [128, 128]
OFF_ONZ = 1924          # [128, 2]
CB16_W = 1926


def _host_consts(w_q, w_k, w_v, w_sigma, w_alpha):
    cb16 = np.zeros((128, CB16_W), BF16)
    wqkv = np.concatenate([w_q, w_k, w_v], axis=1)
    for j in range(6):
        cb16[:, OFF_WEXT + j * 192:OFF_WEXT + (j + 1) * 192] = \
            wqkv[j * 128:(j + 1) * 128].astype(BF16)
    cb16[0:64, OFF_WSA:OFF_WSA + 2] = w_sigma.astype(BF16)
    cb16[0:64, OFF_WSA + 2] = (-w_alpha[:, 0]).astype(BF16)

    sel = np.zeros((FW, NCH * 128), np.float32)
    for c in range(NCH):
        for r in range(TSZ[c]):
            t = TOFF[c] + r
            if t == 0:
                sel[49, c * 128 + r] = 1.0
                continue
            p = t - 1
            sel[p // GRID, c * 128 + r] = 1.0
            sel[24, c * 128 + r] = 1.0
            sel[25 + p % GRID, c * 128 + r] = 1.0
    cb16[0:FW, OFF_SEL:OFF_SEL + NCH * 128] = sel.astype(BF16)
    cb16[:, OFF_ID:OFF_ID + 128] = np.eye(128, dtype=BF16)
    cb16[:, OFF_ONZ] = 1.0

    dxy2s = np.zeros((128, NCH * FW), np.float32)
    for c in range(NCH):
        for r in range(TSZ[c]):
            t = TOFF[c] + r
            o = c * FW
            dxy2s[r, o + 49] = NEG
            if t == 0:
                dxy2s[r, o:o + 24] = NEG
                dxy2s[r, o + 25:o + 49] = NEG
                continue
            p = t - 1
            py, px = p // GRID, p % GRID
            j = np.arange(GRID, dtype=np.float32)
            dxy2s[r, o:o + 24] = -0.5 * (py - j) ** 2
            dxy2s[r, o + 25:o + 49] = -0.5 * (px - j) ** 2
    return cb16, dxy2s


def _trace(nc, tc, ctx, consts_f, need_gb):
    import concourse.mybir as mybir

    dt = mybir.dt
    AF = mybir.ActivationFunctionType
    OP = mybir.AluOpType
    bs0, bs1, ba0 = consts_f[:3]

    xT_d = nc.dram_tensor("xT", [BPC, E, T], dt.bfloat16, kind="ExternalInput").ap()
    cb16_d = nc.dram_tensor("cb16", [128, CB16_W], dt.bfloat16,
                            kind="ExternalInput").ap()
    cf32_d = nc.dram_tensor("cf32", [128, NCH * FW], dt.float32,
                            kind="ExternalInput").ap()
    if need_gb:
        gb_d = nc.dram_tensor("gb", [4, 128, 64], dt.float32, kind="ExternalInput").ap()
    out_d = nc.dram_tensor("out", [BPC, T, H], dt.float32, kind="ExternalOutput").ap()

    cpool = ctx.enter_context(tc.tile_pool(name="consts", bufs=1))
    xpool = ctx.enter_context(tc.tile_pool(name="x", bufs=6))
    wkpool = ctx.enter_context(tc.tile_pool(name="work", bufs=6))
    rpool = ctx.enter_context(tc.tile_pool(name="raw", bufs=12))
    qkpool = ctx.enter_context(tc.tile_pool(name="qk", bufs=4))
    vpool = ctx.enter_context(tc.tile_pool(name="v", bufs=25))
    apool = ctx.enter_context(tc.tile_pool(name="attn", bufs=3))
    spool = ctx.enter_context(tc.tile_pool(name="small", bufs=4))
    opool = ctx.enter_context(tc.tile_pool(name="outb", bufs=3))

    # PSUM: 8 banks total
    ps_sm = ctx.enter_context(tc.tile_pool(name="ps_sm", bufs=2, space="PSUM"))
    ps_pz = ctx.enter_context(tc.tile_pool(name="ps_pz", bufs=2, space="PSUM"))
    ps_pl = ctx.enter_context(tc.tile_pool(name="ps_pl", bufs=2, space="PSUM"))
    ps_zb = ctx.enter_context(tc.tile_pool(name="ps_zb", bufs=1, space="PSUM"))
    ps_po = ctx.enter_context(tc.tile_pool(name="ps_po", bufs=1, space="PSUM"))

    cb16 = cpool.tile([128, CB16_W], dt.bfloat16)
    nc.sync.dma_start(cb16[:], cb16_d)
    dxy2s = cpool.tile([128, NCH * FW], dt.float32)
    nc.sync.dma_start(dxy2s[:], cf32_d)
    w_ext = cb16[:, OFF_WEXT:OFF_WEXT + 6 * 192]
    w_sa = cb16[0:64, OFF_WSA:OFF_WSA + 4]
    sel = cb16[0:FW, OFF_SEL:OFF_SEL + NCH * 128]
    ident = cb16[:, OFF_ID:OFF_ID + 128]
    onz = cb16[:, OFF_ONZ:OFF_ONZ + 2]
    gb = None
    if need_gb:
        gb = cpool.tile([128, 4, 64], dt.float32)
        nc.sync.dma_start(gb[:], gb_d.rearrange("g p h -> p g h"))

    _bias_cache = {}

    def fbias(val, tsz=128):
        val = float(val)
        if val == 0.0:
            return 0.0
        if val not in _bias_cache:
            bt = cpool.tile([128, 1], dt.float32, name=f"bias{len(_bias_cache)}")
            nc.vector.memset(bt[:], val)
            _bias_cache[val] = bt
        return _bias_cache[val][0:tsz, :]

    def phase0(S, b):
        """prefetch x one slot ahead of phase1."""
        S["xt"] = xt = xpool.tile([128, 6, T], dt.bfloat16, tag="xT", name=f"x{b}")
        xsrc = xT_d[b].rearrange("(j p) t -> p j t", p=128)
        if b == 0:
            for j in range(6):
                nc.sync.dma_start(xt[:, j], xsrc[:, j])
        else:
            nc.sync.dma_start(xt[:], xsrc)

    def phase1(S, b):
        """QKV matmuls, raw evac, LN stats (DVE)."""
        xt = S["xt"]
        S["raw"] = raw = [
            rpool.tile([128, 192], dt.bfloat16, tag="raw", name=f"raw{b}_{i}")
            for i in range(NCH)]
        S["mv"] = mv = spool.tile([128, 20], dt.float32, tag="mv", name=f"mv{b}")
        for c in range(NCH):
            tsz, toff = TSZ[c], TOFF[c]
            pqkv = ps_sm.tile([128, 192], dt.float32, tag="sm", name=f"qkv{b}_{c}")
            for j in range(6):
                nc.tensor.matmul(
                    pqkv[0:tsz, 0:192], xt[:, j, toff:toff + tsz],
                    w_ext[:, j * 192:(j + 1) * 192],
                    start=(j == 0), stop=(j == 5),
                )
            nc.vector.tensor_copy(raw[c][0:tsz, :], pqkv[0:tsz, 0:192])
            st = spool.tile([128, 12], dt.float32, tag="st", name=f"st{b}_{c}")
            nc.vector.bn_stats(st[0:tsz, 0:6], raw[c][0:tsz, 0:64])
            nc.vector.bn_stats(st[0:tsz, 6:12], raw[c][0:tsz, 64:128])
            nc.vector.bn_aggr(mv[0:tsz, 4 * c:4 * c + 2], st[0:tsz, 0:6])
            nc.vector.bn_aggr(mv[0:tsz, 4 * c + 2:4 * c + 4], st[0:tsz, 6:12])
            yield

    def phase2(S, b):
        """LN params + apply into 128-aligned blocks, v build (Pool), then
        two blocked DMA transposes produce contiguous qT / kT assemblies."""
        mv = S["mv"]
        raw = S["raw"]
        mv4 = mv[:].rearrange("p (c f) -> p c f", f=4)
        lnv = spool.tile([128, 10], dt.float32, tag="lnv", name=f"lnv{b}")
        lnv2 = lnv[:].rearrange("p (c f) -> p c f", f=2)
        nc.scalar.activation(lnv2[:, :, :], mv4[:, :, 1::2], AF.Ln, bias=fbias(EPS))
        sc = spool.tile([128, 10], dt.float32, tag="sc", name=f"sc{b}")
        nc.scalar.activation(sc[:], lnv[:], AF.Exp, scale=-0.5)
        sc2 = sc[:].rearrange("p (c f) -> p c f", f=2)
        nmr = spool.tile([128, 10], dt.float32, tag="nmr", name=f"nmr{b}")
        nmr2 = nmr[:].rearrange("p (c f) -> p c f", f=2)
        nc.gpsimd.tensor_mul(nmr2[:, :, :], mv4[:, :, 0::2], sc2[:, :, :])
        yield

        qall = qkpool.tile([128, 640], dt.bfloat16, tag="qall", name=f"qall{b}")
        kall = qkpool.tile([128, 640], dt.bfloat16, tag="kall", name=f"kall{b}")
        S["v_ext"] = v_ext = [
            vpool.tile([128, 66], dt.bfloat16, tag="vext", name=f"vext{b}_{i}")
            for i in range(NCH)]
        for c in range(NCH):
            tsz = TSZ[c]
            nc.vector.tensor_scalar(
                qall[0:tsz, 128 * c:128 * c + 64], raw[c][0:tsz, 0:64],
                sc[0:tsz, 2 * c:2 * c + 1], nmr[0:tsz, 2 * c:2 * c + 1],
                OP.mult, OP.subtract)
            nc.vector.tensor_scalar(
                kall[0:tsz, 128 * c:128 * c + 64], raw[c][0:tsz, 64:128],
                sc[0:tsz, 2 * c + 1:2 * c + 2], nmr[0:tsz, 2 * c + 1:2 * c + 2],
                OP.mult, OP.subtract)
            if need_gb:
                gbf = gb[:].rearrange("p g h -> p (g h)")
                nc.vector.tensor_mul(qall[0:tsz, 128 * c:128 * c + 64],
                                     qall[0:tsz, 128 * c:128 * c + 64],
                                     gbf[0:tsz, 0:64])
                nc.vector.tensor_add(qall[0:tsz, 128 * c:128 * c + 64],
                                     qall[0:tsz, 128 * c:128 * c + 64],
                                     gbf[0:tsz, 64:128])
                nc.vector.tensor_mul(kall[0:tsz, 128 * c:128 * c + 64],
                                     kall[0:tsz, 128 * c:128 * c + 64],
                                     gbf[0:tsz, 128:192])
                nc.vector.tensor_add(kall[0:tsz, 128 * c:128 * c + 64],
                                     kall[0:tsz, 128 * c:128 * c + 64],
                                     gbf[0:tsz, 192:256])
            nc.gpsimd.tensor_copy(
                v_ext[c][0:tsz, 0:64], raw[c][0:tsz, 128:192])
            nc.gpsimd.tensor_copy(v_ext[c][0:tsz, 64:66], onz[0:tsz, :])
            yield
        # blocked transpose: out[:, g, :] = in[:, 128g:128(g+1)].T, and the
        # 128-aligned chunk offsets make the result contiguous in t
        S["qT"] = qT = qkpool.tile([128, 640], dt.bfloat16, tag="qT",
                                   name=f"qT{b}")
        S["kT"] = kT = qkpool.tile([128, 640], dt.bfloat16, tag="kT",
                                   name=f"kT{b}")
        nc.sync.dma_start_transpose(
            qT[:].rearrange("p (g f) -> p g f", f=128), qall[:])
        nc.sync.dma_start_transpose(
            kT[:].rearrange("p (g f) -> p g f", f=128), kall[:])
        yield

    def phase3(S, b):
        """sigma/alpha MLP, log-space factor build, blocked DMA transpose."""
        qT = S["qT"]
        psa = ps_sm.tile([128, 20], dt.float32, tag="sm", name=f"psa{b}")
        for c in range(NCH):
            nc.tensor.matmul(
                psa[0:TSZ[c], 4 * c:4 * c + 4], qT[0:64, TOFF[c]:TOFF[c] + TSZ[c]],
                w_sa[:], start=True, stop=True)
        sap = spool.tile([128, 20], dt.float32, tag="sap", name=f"sap{b}")
        nc.vector.tensor_copy(sap[:], psa[:, 0:20])
        yield
        sap4 = sap[:].rearrange("p (c f) -> p c f", f=4)
        # texp = exp(-(q.w_sigma + b_sigma)); spe = exp(q.w_alpha + b_alpha)
        # (w_alpha negated on host so both use scale=-1)
        texp = spool.tile([128, 15], dt.float32, tag="texp", name=f"texp{b}")
        texp3 = texp[:].rearrange("p (c f) -> p c f", f=3)
        if bs0 == bs1 == -ba0:
            nc.scalar.activation(texp3[:, :, :], sap4[:, :, 0:3], AF.Exp,
                                 bias=fbias(-bs0), scale=-1.0)
        else:
            if bs0 == bs1:
                nc.scalar.activation(texp3[:, :, 0:2], sap4[:, :, 0:2], AF.Exp,
                                     bias=fbias(-bs0), scale=-1.0)
            else:
                for col in range(2):
                    nc.scalar.activation(
                        texp3[:, :, col], sap4[:, :, col], AF.Exp,
                        bias=fbias(-(bs0 if col == 0 else bs1)), scale=-1.0)
            nc.scalar.activation(texp3[:, :, 2], sap4[:, :, 2], AF.Exp,
                                 bias=fbias(ba0), scale=-1.0)
        ab = spool.tile([128, 10], dt.float32, tag="ab", name=f"ab{b}")
        ab3 = ab[:].rearrange("p (c f) -> p c f", f=2)
        nc.gpsimd.tensor_add(
            ab3[:, :, :], texp3[:, :, 0:2],
            fbias(1.0).unsqueeze(-1).broadcast_to([128, NCH, 2]))
        nc.gpsimd.tensor_mul(ab[:], ab[:], ab[:])
        spl = spool.tile([128, 5], dt.float32, tag="spl", name=f"spl{b}")
        nc.scalar.activation(spl[:], texp3[:, :, 2], AF.Ln, bias=fbias(1.0))
        lna = spool.tile([128, 5], dt.float32, tag="lna", name=f"lna{b}")
        nc.scalar.activation(lna[:], spl[:], AF.Ln)

        yn = spool.tile([128, NCH * 128], dt.bfloat16, tag="yn", name=f"yn{b}")
        dxy3 = dxy2s[:].rearrange("p (c f) -> p c f", f=FW)
        yn3 = yn[:].rearrange("p (c f) -> p c f", f=128)
        nc.gpsimd.tensor_mul(
            yn3[:, :, 0:24], dxy3[:, :, 0:24],
            ab3[:, :, 1:2].broadcast_to([128, NCH, 24]))
        nc.gpsimd.tensor_mul(
            yn3[:, :, 25:50], dxy3[:, :, 25:50],
            ab3[:, :, 0:1].broadcast_to([128, NCH, 25]))
        nc.gpsimd.tensor_copy(yn3[:, :, 24:25], lna[:].unsqueeze(-1))
        yield
        S["yT"] = yT = wkpool.tile([128, 640], dt.bfloat16, tag="yT",
                                   name=f"yT{b}")
        nc.sync.dma_start_transpose(
            yT[:].rearrange("p (g f) -> p g f", f=128), yn[:])
        yield

    def phase4(S, b):
        """main loop: bias path for all chunks first, then sim^T + exp."""
        qT, kT, yT = S["qT"], S["kT"], S["yT"]
        S["attnT"] = attnT = apool.tile([128, NCH * 578], dt.bfloat16,
                                        tag="attnT", name=f"attnT{b}")
        pzb = ps_zb.tile([128, 330], dt.float32, tag="zb", name=f"pzb{b}")
        expls = []
        for c in range(NCH):
            tsz = TSZ[c]
            qlo, wae, w, tail = QLO[c], WAE[c], WB[c], TAILB[c]
            wt = w + (66 if tail else 0)
            selc = sel[:, c * 128:c * 128 + tsz]
            pl = ps_pl.tile([128, 512], dt.float32, tag="pl", name=f"pl{b}_{c}")
            nc.tensor.matmul(pl[0:tsz, 0:w], selc, yT[0:FW, qlo:wae],
                             start=True, stop=True)
            if tail:
                nc.tensor.matmul(pl[0:tsz, w:wt], selc, yT[0:FW, 512:578],
                                 start=True, stop=True)
            expl = wkpool.tile([128, 386], dt.bfloat16, tag="expl",
                               name=f"expl{b}_{c}")
            nc.scalar.activation(expl[0:tsz, 0:wt], pl[0:tsz, 0:wt], AF.Exp)
            expls.append(expl)
            yield
        for c in range(NCH):
            tsz, toff = TSZ[c], TOFF[c]
            qlo, wae, w, tail = QLO[c], WAE[c], WB[c], TAILB[c]
            wt = w + (66 if tail else 0)
            kTc = kT[0:64, toff:toff + tsz]
            expl = expls[c]
            pza = ps_pz.tile([128, 512], dt.float32, tag="pz", name=f"pz{b}_{c}")
            if qlo > 0:
                nc.tensor.matmul(pza[0:tsz, 0:qlo], kTc, qT[0:64, 0:qlo],
                                 start=True, stop=True)
            nc.tensor.matmul(pza[0:tsz, qlo:wae], kTc, qT[0:64, qlo:wae],
                             start=True, stop=False)
            nc.tensor.matmul(pza[0:tsz, qlo:wae], ident[0:tsz, 0:tsz],
                             expl[0:tsz, 0:w], start=False, stop=True)
            if wae < 512:
                nc.tensor.matmul(pza[0:tsz, wae:512], kTc, qT[0:64, wae:512],
                                 start=True, stop=True)
            nc.scalar.activation(attnT[0:tsz, 578 * c:578 * c + 512],
                                 pza[0:tsz, :], AF.Exp, scale=0.125)
            nc.tensor.matmul(pzb[0:tsz, 66 * c:66 * c + 66], kTc,
                             qT[0:64, 512:578], start=True, stop=not tail)
            if tail:
                nc.tensor.matmul(pzb[0:tsz, 66 * c:66 * c + 66],
                                 ident[0:tsz, 0:tsz], expl[0:tsz, w:wt],
                                 start=False, stop=True)
            yield
        # one act finishes all 5 chunks' tail columns
        at3 = attnT[:].rearrange("p (c q) -> p c q", q=578)
        nc.scalar.activation(at3[:, :, 512:578],
                             pzb[:].rearrange("p (c f) -> p c f", f=66),
                             AF.Exp, scale=0.125)

    def phase5(S, b):
        """attn @ [v|1] + batched normalize + store (DVE queue)."""
        attnT, v_ext = S["attnT"], S["v_ext"]
        po = ps_po.tile([128, 330], dt.float32, tag="po", name=f"po{b}")
        for qc in range(NCH):
            qsz, qoff = TSZ[qc], TOFF[qc]
            for kc in range(NCH):
                nc.tensor.matmul(
                    po[0:qsz, 66 * qc:66 * qc + 66],
                    attnT[0:TSZ[kc], 578 * kc + qoff:578 * kc + qoff + qsz],
                    v_ext[kc][0:TSZ[kc], :], start=(kc == 0), stop=(kc == 4))
            yield
        po5 = po[:].rearrange("p (c f) -> p c f", f=66)
        rcp = spool.tile([128, 5], dt.float32, tag="rcp", name=f"rcp{b}")
        nc.vector.reciprocal(rcp[:, :], po5[:, :, 64:65])
        osb = opool.tile([128, 320], dt.float32, tag="osb", name=f"osb{b}")
        osb3 = osb[:].rearrange("p (c f) -> p c f", f=64)
        nc.vector.scalar_tensor_tensor(
            osb3, po5[:, :, 0:64], 1.0,
            rcp[:].unsqueeze(-1).broadcast_to([128, NCH, 64]),
            OP.mult, OP.mult)
        nc.sync.dma_start(
            out_d[b, 0:512, :].rearrange("(c p) h -> p c h", p=128),
            osb[:, 0:256].rearrange("p (c h) -> p c h", h=64))
        nc.sync.dma_start(out_d[b, 512:T, :], osb[0:65, 256:320])

    # anti-diagonal software pipeline across batches; within a slot the five
    # active phases are emitted round-robin at chunk granularity so each
    # engine's in-order queue alternates between independent batches
    # (blocked instructions park in the 4-deep wait queue while ready ones
    # behind them dispatch). x is prefetched 2 slots ahead of its phase1.
    gen_phases = [phase1, phase2, phase3, phase4, phase5]
    offs = [1, 2, 3, 4, 5]
    states = [dict() for _ in range(BPC)]
    for slot in range(BPC + 5):
        if 0 <= slot < BPC:
            phase0(states[slot], slot)
        gens = []
        for p, off in zip(gen_phases, offs):
            bb = slot - off
            if 0 <= bb < BPC:
                gens.append(p(states[bb], bb))
        while gens:
            nxt = []
            for g in gens:
                try:
                    next(g)
                    nxt.append(g)
                except StopIteration:
                    pass
            gens = nxt


_CACHE = {}


def _patch_act_tables():
    # bacc's insert_act_table_loads maps each activation func to the first
    # table containing it, which makes Exp<->Ln transitions reload tables
    # (1.28 us each, ~30x per kernel). Restrict the funcs this kernel uses
    # to the combined natural_log_exp_and_others set so one load suffices.
    import concourse.bacc as bacc_mod
    import concourse.mybir as mybir
    from concourse.hw_specs import get_activation_tables as _gat
    if getattr(bacc_mod, "_ant_act_tables_patched", False):
        return
    AF = mybir.ActivationFunctionType
    mine = {AF.Exp, AF.Ln, AF.Copy, AF.Identity, AF.MemsetZero}

    def patched(arch):
        tabs = _gat(arch)
        combined = tabs.get("natural_log_exp_and_others")
        if combined and mine <= combined:
            for name, s in tabs.items():
                if name != "natural_log_exp_and_others":
                    tabs[name] = s - mine
        return tabs

    bacc_mod.get_activation_tables = patched
    bacc_mod._ant_act_tables_patched = True


def _build(consts_f, need_gb):
    import concourse.tile as tile
    from concourse import bacc

    _patch_act_tables()
    key = (consts_f, need_gb)
    if key in _CACHE:
        return _CACHE[key]
    nc = bacc.Bacc("TRN2", target_bir_lowering=False, debug=False)
    with tile.TileContext(nc) as tc, ExitStack() as ctx:
        _trace(nc, tc, ctx, consts_f, need_gb)
    nc.finalize()
    _CACHE[key] = nc
    return nc


def kernel(x, w_q, w_k, w_v, q_gamma, q_beta, k_gamma, k_beta,
           w_sigma, b_sigma, w_alpha, b_alpha):
    from concourse import bass_utils

    x = np.asarray(x, np.float32)
    w_q, w_k, w_v = (np.asarray(a, np.float32) for a in (w_q, w_k, w_v))
    w_sigma = np.asarray(w_sigma, np.float32)
    w_alpha = np.asarray(w_alpha, np.float32)
    b_sigma = np.asarray(b_sigma, np.float32)
    b_alpha = np.asarray(b_alpha, np.float32)
    q_gamma, q_beta = np.asarray(q_gamma, np.float32), np.asarray(q_beta, np.float32)
    k_gamma, k_beta = np.asarray(k_gamma, np.float32), np.asarray(k_beta, np.float32)

    trivial_gb = (
        np.allclose(q_gamma, 1) and np.allclose(k_gamma, 1)
        and np.allclose(q_beta, 0) and np.allclose(k_beta, 0)
    )

    cb16, cf32 = _host_consts(w_q, w_k, w_v, w_sigma, w_alpha)
    consts_f = (float(b_sigma[0]), float(b_sigma[1]), float(b_alpha[0]))
    nc = _build(consts_f, not trivial_gb)

    xt = np.ascontiguousarray(
        x.reshape(NCORES, BPC, T, E).transpose(0, 1, 3, 2)).astype(BF16)

    base = {"cb16": cb16, "cf32": cf32}
    if not trivial_gb:
        base["gb"] = np.ascontiguousarray(np.broadcast_to(
            np.stack([q_gamma, q_beta, k_gamma, k_beta])[:, None, :],
            (4, 128, 64))).astype(np.float32)
    in_maps = [{**base, "xT": xt[c]} for c in range(NCORES)]

    res = bass_utils.run_bass_kernel_spmd(nc, in_maps, core_ids=list(range(NCORES)))
    out = np.concatenate([res.results[c]["out"] for c in range(NCORES)], axis=0)
    return out.astype(np.float32)
